# revision 1
# baseline (speedup 1.0000x reference)
"""DRNL filterbank Trainium2 kernel.

Strategy: the three IIR cascades (linear path, nonlinear pre-compression,
nonlinear post-compression) are LTI filters applied with zero initial
state, so each equals a causal convolution with its (decaying) impulse
response.  We precompute the impulse responses on the host in float64
from the SOS coefficient inputs (deterministic filter constants),
truncate per-filter at a ~1e-5 tail-energy tolerance, and run the
convolutions on the TensorEngine as Toeplitz-block float32r matmuls
(full-rate fp32 path, ~1.6e-4 end-to-end noise floor).  The broken-stick
compression between the two nonlinear-path convolutions runs on the
Vector/Scalar engines as

    comp = y1 * min(a, exp(-0.75*ln(ya) + ln b)),  ya = |y1|
         ( = sign(y1) * min(a*ya, b*ya^0.25) )

with the Ln/Exp/Abs LUTs pinned to one activation table set.

Sharding: 50 filters -> 8 cores x 7 slots (6 dummy zero slots), slots
sized by the per-band max Toeplitz block count.  Each core processes all
8 batches x 16000 samples for its filters.  Convolution matmul:
out[r, (m,b)] += sum_s W_d[s,r] * X[s, b, m-d], W_d[s,r] = h[d*128+r-s];
X[s, b, m] = x[b, m*128+s] arrives pre-transposed/pre-padded from the
host.  Dummy PE matmuls at the head burn the clock-ramp p-state while
the input DMAs land.
"""

import numpy as np

import concourse.bacc as bacc
import concourse.tile as tile
import concourse.mybir as mybir
from concourse import bass_utils

F32 = mybir.dt.float32
F32R = mybir.dt.float32r
U32 = mybir.dt.uint32
MM_DT = F32R  # matmul streaming dtype: float32r = full-rate fp32 path

T = 16000
B = 8
NF = 50
NCORES = 8
NSLOT = 7  # ceil(50/8)
M = T // 128  # 125 time tiles
COMP_C = 0.25
# tail-energy tolerances; nl1 is looser because the broken-stick compression
# attenuates relative error in y1 by 4x in the compressed (dominant) regime,
# and all truncation stays below the ~1.5e-4 fp32r matmul noise floor
TAIL_TOL = 2e-5
TAIL_TOL_NL1 = 4e-5
NIR = 4096  # max impulse response length considered (=> J <= 32)
# processing order of slots: mid-size first (weights stream in fast enough
# to keep PE fed), biggest last (its lin conv hides its compression chain)
SLOT_ORDER = [4, 2, 3, 1, 5, 6, 0]

_CACHE = {}
_ACT_PIN = None  # (json_path, patched_fn) once prepared


def _prepare_act_pin():
    """Prepare the pinned natural_log_exp_and_others activation table.

    The default per-instruction set choice picks the first set containing
    each function, so interleaved Abs/Ln/Exp/Copy ping-pongs between two
    table sets and pays a ~1.3us table load each time.  One set contains
    all four functions; pin it (both for bacc's table-load placement pass
    and for walrus via BASS_ACT_ROOT_JSON_PATH).
    """
    global _ACT_PIN
    if _ACT_PIN is not None:
        return _ACT_PIN
    import json
    import os
    import shutil
    import tempfile

    from neuronxcc.driver.Job import Job
    from neuronxcc.driver.jobs.support.FindActInfo import findActInfoFile

    src = findActInfoFile(Job.getPackageDir(), "gen3")
    src_dir = os.path.dirname(src)
    info = json.load(open(src))
    ent = [e for e in info["act_func_sets"]
           if e["name"] == "natural_log_exp_and_others"][0]
    tdir = tempfile.mkdtemp(prefix="actpin_")
    for k in info["pwp_file_keys"]:
        shutil.copy(os.path.join(src_dir, ent[k]), os.path.join(tdir, ent[k]))
    jpath = os.path.join(tdir, "act_info.json")
    with open(jpath, "w") as f:
        json.dump({"pwp_file_keys": info["pwp_file_keys"],
                   "act_func_sets": [ent]}, f)

    funcs = {mybir.ActivationFunctionType.from_pwp(v) for v in ent["act"]}
    tables = {"natural_log_exp_and_others": funcs}

    def patched(module_arch):
        return tables

    _ACT_PIN = (jpath, patched)
    return _ACT_PIN


class _pinned_act_tables:
    """Scoped activation-table pin: env var + get_activation_tables patch."""

    def __enter__(self):
        import os
        import concourse.hw_specs as hw_specs
        import concourse.bacc as bacc_mod
        import concourse.bass_interp as bass_interp
        jpath, patched = _prepare_act_pin()
        self._mods = (hw_specs, bacc_mod, bass_interp)
        self._saved = [m.get_activation_tables for m in self._mods]
        self._env_old = os.environ.get("BASS_ACT_ROOT_JSON_PATH")
        os.environ["BASS_ACT_ROOT_JSON_PATH"] = jpath
        for m in self._mods:
            m.get_activation_tables = patched
        return self

    def __exit__(self, *exc):
        import os
        for m, fn in zip(self._mods, self._saved):
            m.get_activation_tables = fn
        if self._env_old is None:
            os.environ.pop("BASS_ACT_ROOT_JSON_PATH", None)
        else:
            os.environ["BASS_ACT_ROOT_JSON_PATH"] = self._env_old
        return False


# ---------------------------------------------------------------- host math

def _cascade_irs(sos_list, nir):
    """Impulse responses of biquad cascades.

    sos_list: list of [NF, K, 6] arrays. Returns list of [NF, nir] float64.
    """
    out = []
    nfft = 2 * nir
    for sos in sos_list:
        sos = np.asarray(sos, np.float64)
        nf, K, _ = sos.shape
        b0, b1, b2 = sos[:, :, 0], sos[:, :, 1], sos[:, :, 2]
        a1, a2 = sos[:, :, 4], sos[:, :, 5]
        # per-section impulse responses via direct recursion
        h = np.zeros((nir, nf, K))
        hm1 = np.zeros((nf, K))
        hm2 = np.zeros((nf, K))
        for n in range(nir):
            v = -a1 * hm1 - a2 * hm2
            if n == 0:
                v = v + b0
            elif n == 1:
                v = v + b1
            elif n == 2:
                v = v + b2
            h[n] = v
            hm2 = hm1
            hm1 = v
        H = np.fft.rfft(h, nfft, axis=0)  # [nfft/2+1, nf, K]
        Hc = np.prod(H, axis=2)  # cascade product
        hc = np.fft.irfft(Hc, nfft, axis=0)[:nir]  # [nir, nf]
        out.append(np.ascontiguousarray(hc.T))  # [nf, nir]
    return out


def _shifts_needed(h, tol):
    """Per-filter Toeplitz shift count J.

    J shifts cover at least (J-1)*128+1 taps for every output offset
    (offset r covers (J-1)*128+r taps).
    """
    e = np.cumsum(h * h, axis=1)
    tot = e[:, -1:]
    nf = h.shape[0]
    # S blocks guarantee (S-1)*128 + r taps at output offset r.  Require
    # full coverage at every offset of the tap count whose tail is below
    # 2.5*tol (the blocks carry the full response, so most offsets see
    # deeper coverage for free; worst-offset truncation stays ~2.5*tol,
    # below the fp32r matmul noise floor).
    t2 = 2.5 * tol
    need = np.empty(nf, np.int64)
    for f in range(nf):
        need[f] = np.searchsorted(e[f] / tot[f, 0], 1.0 - t2 * t2) + 1
    need = np.minimum(need, h.shape[1])
    return (need + 127) // 128 + 1


def _round_f32r(a):
    """Round fp32 to the FP32R-representable set (bf16 hi/lo pair sum)."""
    import ml_dtypes
    a = np.asarray(a, np.float32)
    hi = a.astype(ml_dtypes.bfloat16).astype(np.float32)
    lo = (a - hi).astype(ml_dtypes.bfloat16).astype(np.float32)
    return hi + lo


def _toeplitz_blocks(h, J):
    """W[d, s, r] = h[d*128 + r - s] for d < J (h zero-padded)."""
    hpad = np.zeros(128 + J * 128, np.float64)
    n = min(J * 128, h.shape[0])
    hpad[128:128 + n] = h[:n]
    d = np.arange(J)[:, None, None] * 128
    s = np.arange(128)[None, :, None]
    r = np.arange(128)[None, None, :]
    return hpad[128 + d + r - s].astype(np.float32)


# ---------------------------------------------------------------- device

def _build_program(J1, JL, J2, slot_order=None, wchunk_size=8,
                   abs_on_dve=False, comp_bufs=3, tmp_bufs=6, st_bufs=3,
                   n_warm=10):
    """Build + compile the shared SPMD program. J*: per-slot block counts."""
    nslot = len(J1)
    if slot_order is None:
        slot_order = list(range(nslot))
    sJ1, sJL, sJ2 = sum(J1), sum(JL), sum(J2)
    o1 = np.cumsum([0] + list(J1))
    oL = np.cumsum([0] + list(JL))
    o2 = np.cumsum([0] + list(J2))
    JX = max(max(J1), max(JL))
    JC = max(J2)

    ctx = _pinned_act_tables()
    ctx.__enter__()
    try:
        nc = _build_body(J1, JL, J2, slot_order, wchunk_size, abs_on_dve,
                         comp_bufs, tmp_bufs, st_bufs, nslot,
                         sJ1, sJL, sJ2, o1, oL, o2, JX, JC, n_warm)
    finally:
        ctx.__exit__()
    return nc


def _build_body(J1, JL, J2, slot_order, wchunk_size, abs_on_dve,
                comp_bufs, tmp_bufs, st_bufs, nslot,
                sJ1, sJL, sJ2, o1, oL, o2, JX, JC, n_warm):
    nc = bacc.Bacc("TRN2", target_bir_lowering=False, debug=False,
                   num_devices=NCORES)

    xt_d = nc.dram_tensor("xt", [128, B, JX + M], MM_DT,
                          kind="ExternalInput").ap()
    w1_d = nc.dram_tensor("w1", [128, sJ1, 128], MM_DT, kind="ExternalInput").ap()
    wl_d = nc.dram_tensor("wl", [128, sJL, 128], MM_DT, kind="ExternalInput").ap()
    w2_d = nc.dram_tensor("w2", [128, sJ2, 128], MM_DT, kind="ExternalInput").ap()
    ab_d = nc.dram_tensor("ab", [128, nslot * 2], F32, kind="ExternalInput").ap()
    out_d = nc.dram_tensor("out", [nslot, 128, 2 * 500], F32,
                           kind="ExternalOutput").ap()

    with tile.TileContext(nc) as tc:
        with (
            tc.tile_pool(name="const", bufs=1) as cpool,
            tc.tile_pool(name="comp", bufs=comp_bufs) as compool,
            tc.tile_pool(name="tmp", bufs=tmp_bufs) as tpool,
            tc.tile_pool(name="stage", bufs=st_bufs) as stpool,
            tc.tile_pool(name="ps", bufs=8, space="PSUM") as psp,
        ):
            w1_sb = cpool.tile([128, sJ1 * 128], MM_DT)
            wl_sb = cpool.tile([128, sJL * 128], MM_DT)
            w2_sb = cpool.tile([128, sJ2 * 128], MM_DT)
            ab_sb = cpool.tile([128, nslot * 2], F32)
            xbuf = cpool.tile([128, B, JX + M], MM_DT)

            # x arrives pre-transposed and pre-padded from the host:
            # xt[s, b, JX+m] = x[b, m*128+s], zeros in the first JX columns
            nc.sync.dma_start(xbuf[:], xt_d)

            # PE warmup: burn the clock-ramp p-state on dummy matmuls while
            # the input DMAs land, so the real convolutions start at full rate
            wz = cpool.tile([128, 512], MM_DT)
            nc.gpsimd.memset(wz[:].bitcast(U32), 0)
            pwarm = psp.tile([128, 512], F32, tag="ps")
            for _ in range(n_warm):
                nc.tensor.matmul(pwarm[:, 0:500], wz[:, 0:128], wz[:, 0:500],
                                 start=True, stop=True)

            nc.scalar.dma_start(ab_sb[:], ab_d)
            # weights arrive chunked per (slot, conv) in processing order,
            # so the first matmuls don't wait for the whole weight transfer
            for s in slot_order:
                for w_sb, w_d, off, J in (
                    (w1_sb, w1_d, o1[s], J1[s]),
                    (wl_sb, wl_d, oL[s], JL[s]),
                    (w2_sb, w2_d, o2[s], J2[s]),
                ):
                    for c0 in range(0, J, wchunk_size):
                        cj = min(wchunk_size, J - c0)
                        nc.sync.dma_start(
                            w_sb[:, (off + c0) * 128:(off + c0 + cj) * 128],
                            w_d[:, off + c0:off + c0 + cj, :].rearrange(
                                "s j r -> s (j r)"))

            def conv(ps_h, srcs, h, w_sb, off, J, JP, start, stop):
                src_h = srcs[h] if isinstance(srcs, tuple) else \
                    srcs[:, 4 * h:4 * h + 4, :]
                for d in range(J):
                    nc.tensor.matmul(
                        ps_h[:, 0:500],
                        w_sb[:, (off + d) * 128:(off + d + 1) * 128],
                        src_h[:, 0:4,
                              JP - d:JP - d + M].rearrange("p b m -> p m b"),
                        start=start and d == 0,
                        stop=stop and d == J - 1,
                    )

            for s in slot_order:
                # ---- nl1 convolution ----
                pa0 = psp.tile([128, 512], F32, tag="ps")
                pa1 = psp.tile([128, 512], F32, tag="ps")
                pa = (pa0, pa1)
                for h in range(2):
                    conv(pa[h], xbuf, h, w1_sb, o1[s], J1[s], JX, True, True)

                # ---- broken-stick compression ----
                # comp = y1 * min(a, b*ya^(-3/4)),  ya = max(|y1|, 1e-12)
                compb0 = compool.tile([128, 4, JC + M], MM_DT, tag="compb0")
                compb1 = compool.tile([128, 4, JC + M], MM_DT, tag="compb1")
                compbs = (compb0, compb1)
                nc.gpsimd.memset(compb0[:, :, 0:JC].bitcast(U32), 0)
                nc.gpsimd.memset(compb1[:, :, 0:JC].bitcast(U32), 0)
                a_ap = ab_sb[:, 2 * s:2 * s + 1]
                lnb_ap = ab_sb[:, 2 * s + 1:2 * s + 2]
                # fully per-half pipeline: h0's chain completes (and nl2-h0
                # can start) without waiting for h1's abs/ln/exp
                for h in range(2):
                    ya = tpool.tile([128, 500], F32, tag="ya", name=f"ya{h}")
                    nc.scalar.activation(
                        ya[:], pa[h][:, 0:500],
                        mybir.ActivationFunctionType.Abs)
                    lg = tpool.tile([128, 500], F32, tag="lg", name=f"lg{h}")
                    nc.scalar.activation(
                        lg[:], ya[:], mybir.ActivationFunctionType.Ln)
                    q = tpool.tile([128, 500], F32, tag="q", name=f"q{h}")
                    nc.scalar.activation(
                        q[:], lg[:], mybir.ActivationFunctionType.Exp,
                        bias=lnb_ap, scale=-0.75)
                    nc.vector.tensor_scalar(
                        q[:], q[:], a_ap, None, op0=mybir.AluOpType.min)
                    nc.vector.tensor_tensor(
                        compbs[h][:, 0:4,
                                  JC:JC + M].rearrange("p b m -> p m b"),
                        q[:], pa[h][:, 0:500],
                        op=mybir.AluOpType.mult)

                # ---- lin + nl2 convolutions, accumulated ----
                pb0 = psp.tile([128, 512], F32, tag="ps")
                pb1 = psp.tile([128, 512], F32, tag="ps")
                pb = (pb0, pb1)
                st = stpool.tile([128, 1000], F32)
                for h in range(2):
                    conv(pb[h], xbuf, h, wl_sb, oL[s], JL[s], JX, True, False)
                    conv(pb[h], compbs, h, w2_sb, o2[s], J2[s], JC, False, True)
                    # evict + store per half so h0's output overlaps h1's convs
                    if h == 0:
                        nc.scalar.copy(st[:, 0:500], pb[0][:, 0:500])
                    else:
                        nc.vector.tensor_copy(st[:, 500:1000], pb[1][:, 0:500])
                    nc.sync.dma_start(out_d[s][:, 500 * h:500 * h + 500],
                                      st[:, 500 * h:500 * h + 500])

    nc.compile()
    return nc


# ---------------------------------------------------------------- driver

def _prepare(g_lin, a_nl, b_nl, sos_lin, sos_nl1, sos_nl2):
    h_lin, h_nl1, h_nl2 = _cascade_irs([sos_lin, sos_nl1, sos_nl2], NIR)
    h_lin = h_lin * np.asarray(g_lin, np.float64)[:, None]

    jl = _shifts_needed(h_lin, TAIL_TOL)
    j1 = _shifts_needed(h_nl1, TAIL_TOL_NL1)
    j2 = _shifts_needed(h_nl2, TAIL_TOL)

    # rank filters by total work, slot r//8 on core r%8
    order = np.argsort(-(jl + j1 + j2), kind="stable")
    # per-slot uniform block counts (max over the 8 filters in the slot)
    J1 = []
    JL = []
    J2 = []
    assign = {}  # (core, slot) -> filter
    for s in range(NSLOT):
        ranks = order[s * 8:(s + 1) * 8]
        J1.append(int(max(1, max(j1[f] for f in ranks))))
        JL.append(int(max(1, max(jl[f] for f in ranks))))
        J2.append(int(max(1, max(j2[f] for f in ranks))))
        for c, f in enumerate(ranks):
            assign[(c, s)] = int(f)
    return (h_lin, h_nl1, h_nl2, jl, j1, j2,
            tuple(J1), tuple(JL), tuple(J2), assign)


def _make_inputs(x, a_nl, b_nl, prep):
    (h_lin, h_nl1, h_nl2, jl, j1, j2, J1, JL, J2, assign) = prep
    o1 = np.cumsum([0] + list(J1))
    oL = np.cumsum([0] + list(JL))
    o2 = np.cumsum([0] + list(J2))
    x = np.asarray(x, np.float32)
    # xt[s, b, JX+m] = x[b, m*128 + s] with JX leading zero tiles (the
    # convolution's causal pad), pre-rounded to the fp32r grid
    JX = max(max(J1), max(JL))
    xt = np.zeros((128, B, JX + M), np.float32)
    xt[:, :, JX:] = x.reshape(B, M, 128).transpose(2, 0, 1)
    xt = _round_f32r(xt)

    in_maps = []
    for c in range(NCORES):
        w1 = np.zeros((o1[-1], 128, 128), np.float32)
        wl = np.zeros((oL[-1], 128, 128), np.float32)
        w2 = np.zeros((o2[-1], 128, 128), np.float32)
        # shipped as [s, j, r] so every weight DMA is contiguous per partition
        ab = np.zeros((128, NSLOT * 2), np.float32)
        ab[:, 1::2] = -100.0  # dummy: exp(...)=0
        for s in range(NSLOT):
            f = assign.get((c, s))
            if f is None:
                continue
            w1[o1[s]:o1[s] + J1[s]] = _toeplitz_blocks(h_nl1[f], J1[s])
            wl[oL[s]:oL[s] + JL[s]] = _toeplitz_blocks(h_lin[f], JL[s])
            w2[o2[s]:o2[s] + J2[s]] = _toeplitz_blocks(h_nl2[f], J2[s])
            ab[:, 2 * s] = np.float32(a_nl[f])
            ab[:, 2 * s + 1] = np.float32(np.log(np.float64(b_nl[f])))
        tr = lambda w: np.ascontiguousarray(
            _round_f32r(w).transpose(1, 0, 2))
        in_maps.append({
            "xt": xt, "w1": tr(w1), "wl": tr(wl), "w2": tr(w2), "ab": ab,
        })
    return in_maps


def _gather(results, assign):
    # out[s][r, c]: c = h*500 + m*4 + bh, batch = h*4 + bh, t = m*128 + r
    y = np.zeros((B, NF, T), np.float32)
    for (c, s), f in assign.items():
        o = results[c]["out"].reshape(NSLOT, 128, 2, M, 4)
        y[:, f, :] = o[s].transpose(1, 3, 2, 0).reshape(B, T)
    return y


def kernel(x, g_lin, a_nl, b_nl, sos_lin, sos_nl1, sos_nl2):
    x = np.asarray(x, np.float32)
    g_lin = np.asarray(g_lin, np.float64)
    a_nl = np.asarray(a_nl, np.float64)
    b_nl = np.asarray(b_nl, np.float64)
    sos_lin = np.asarray(sos_lin, np.float64)
    sos_nl1 = np.asarray(sos_nl1, np.float64)
    sos_nl2 = np.asarray(sos_nl2, np.float64)

    key = (sos_lin.tobytes(), sos_nl1.tobytes(), sos_nl2.tobytes(),
           g_lin.tobytes())
    if key not in _CACHE:
        prep = _prepare(g_lin, a_nl, b_nl, sos_lin, sos_nl1, sos_nl2)
        nc = _build_program(prep[6], prep[7], prep[8],
                            slot_order=list(SLOT_ORDER))
        _CACHE[key] = (prep, nc)
    prep, nc = _CACHE[key]

    in_maps = _make_inputs(x, a_nl, b_nl, prep)
    # the walrus/NEFF compile happens lazily on the first execution and
    # reads BASS_ACT_ROOT_JSON_PATH, so keep the table pin active here too
    with _pinned_act_tables():
        res = bass_utils.run_bass_kernel_spmd(nc, in_maps,
                                              core_ids=list(range(NCORES)))
    return _gather(res.results, prep[9])

